# revision 1
# baseline (speedup 1.0000x reference)
"""Trainium2 Bass kernel for DecomposingAttnProcessor (pooled component softmax
cross-attention), sharded over 8 NeuronCores along the latent-token axis S.

Math (per batch-component bc = c*B + b):
    q = x @ Wq ; k = enc @ Wk ; v = enc @ Wv           (per-head, dh = 64)
    scores = (q k^T) * dh^-0.5                          [H, S, E]
    pooled = mean_E scores ; wp = softmax_c(pooled)
    w = softmax_E(scores) * wp
    out = (w v) @ Wo + bo + x

Sharding: each core owns a 512-row slice of S for ALL batch-components; the
component softmax couples only the c axis, which stays on-core.

Dataflow per core (all matmuls contract over the SBUF partition axis):
    xT   via PE transpose;  qT = Wq^T xT (fp32r), scaled by dh^-0.5 on evict
    kT   = Wk^T encT (bf16) with fused column 160 = ksum/E  (for pooled)
    scoresT[t, s] = kT_h^T  qT_h      -> exp on ACT evict (bf16 weights)
    pooled row / denom row per (c,h) via M=1 matmuls into one [64, S] PSUM tile
    coef = softmax_c(exp pooled) / denom, broadcast over dh by GPSIMD
    aoT[dh, s] = v_h^T w  (bf16), scaled in-place by coef
    out = aoT^T @ Wo + bo + x  (bf16 matmul, fp32 residual add)
"""

import sys
from contextlib import ExitStack

sys.path.insert(0, "/opt/trn_rl_repo")

import numpy as np

import concourse.bass as bass  # noqa: E402
from concourse import bacc, mybir  # noqa: E402
from concourse.bass_utils import run_bass_kernel_spmd  # noqa: E402
from concourse.masks import make_identity  # noqa: E402
from concourse.tile import TileContext  # noqa: E402

# Problem dims (hardcoded per spec)
BC, S, D, E, H, C = 8, 4096, 1024, 160, 16, 4
B = BC // C  # 2
DH = D // H  # 64
SCALE = DH**-0.5  # 0.125
N_CORES = 8
S_LOC = S // N_CORES  # 512 rows of S per core
S_TILE = 256  # rows processed per iteration
E0, E1 = 128, E - 128  # encoder-token chunks (128 + 32)
ND = D // 128  # 8 chunks of the hidden dim

F32 = mybir.dt.float32
F32R = mybir.dt.float32r
BF16 = mybir.dt.bfloat16


def build_body(ctx, tc, d, s_loc):
    nc = tc.nc
    P = 128
    n_sc = s_loc // S_TILE

    pools = {}

    def pool(name, bufs, space="SBUF"):
        if name not in pools:
            pools[name] = ctx.enter_context(tc.tile_pool(name=name, bufs=bufs, space=space))
        return pools[name]

    const = pool("const", 1)
    wmat = pool("wmat", 1)  # Wk|Wv bf16 pairs, later reused for Wq f32
    wop = pool("wo", 1)
    enc_in = pool("enc_in", 1)
    enct_p = pool("enct", 1)
    small = pool("small", 1)
    stage = pool("stage", 2)
    ktp = pool("kt", 1)
    vp = pool("v", 1)
    xin_p = pool("xin", 2)
    xt_p = pool("xt", 2)
    qt_p = pool("qt", 2)
    w_p = pool("w", 6)
    ao_p = pool("ao", 4)
    stats = pool("stats", 2)
    coefb_p = pool("coefb", 2)
    denst_p = pool("denst", 2)
    xr_p = pool("xr", 2)
    oh_p = pool("oh", 2)

    ksb_p = pool("ksb", 1)
    dram = pool("dram", 1, space="DRAM")

    psml = pool("psml", 4, space="PSUM")  # f32 matmul accumulators (3 banks)
    pstp = pool("pstp", 1, space="PSUM")  # bf16 transpose targets (2 banks)
    psst = pool("psst", 1, space="PSUM")  # pooled + denom collectors (2 banks)
    pbig = pool("pbig", 1, space="PSUM")  # [128,512] projections (1 bank)

    # ---- constants ----
    ident = const.tile([P, P], BF16, tag="ident")
    make_identity(nc, ident)
    ones_row = const.tile([1, P], BF16, tag="ones_row")
    nc.vector.memset(ones_row, 1.0)
    bo_bf = const.tile([1, D], BF16, tag="bo_bf")
    nc.gpsimd.dma_start(out=bo_bf, in_=d["bo"])  # f32 -> bf16 cast DMA

    # ---- load Wk/Wv as bf16 pairs ----
    wkv = []
    for i in range(ND):
        t = wmat.tile([P, 2 * D], BF16, tag=f"w{i}")
        nc.gpsimd.dma_start(out=t[:, 0:D], in_=d["Wk"][128 * i : 128 * (i + 1), :])
        nc.gpsimd.dma_start(out=t[:, D : 2 * D], in_=d["Wv"][128 * i : 128 * (i + 1), :])
        wkv.append(t)

    wo = []
    for i in range(ND):
        t = wop.tile([P, D], BF16, tag=f"wo{i}")
        nc.gpsimd.dma_start(out=t, in_=d["Wo"][128 * i : 128 * (i + 1), :])
        wo.append(t)

    # ---- DRAM scratch (pool-allocated so Tile tracks spill->reload deps) ----
    kts = [dram.tile([ND, P, E + 1], BF16, tag=f"kts{bc}", name=f"kts{bc}") for bc in range(BC)]
    v0s = [dram.tile([E0, H * (DH + 1)], BF16, tag=f"v0s{bc}", name=f"v0s{bc}") for bc in range(BC)]
    v1s = [dram.tile([E1, H * (DH + 1)], BF16, tag=f"v1s{bc}", name=f"v1s{bc}") for bc in range(BC)]

    # ---- encoder phase: per bc, compute kT (+ksum/E col) and v, spill to DRAM ----
    for bc in range(BC):
        et0 = enc_in.tile([P, D], BF16, tag="et0")
        et1 = enc_in.tile([E1, D], BF16, tag="et1")
        nc.gpsimd.dma_start(out=et0, in_=d["enc"][bc, 0:E0, :])
        nc.gpsimd.dma_start(out=et1, in_=d["enc"][bc, E0:E, :])

        enct = []
        for i in range(ND):
            ps = pstp.tile([P, E], BF16, tag="pst")
            sl = slice(128 * i, 128 * (i + 1))
            nc.tensor.transpose(ps[:, 0:E0], et0[:, sl], ident)
            nc.tensor.transpose(ps[:, E0:E], et1[:, sl], ident[0:E1, 0:E1])
            t = enct_p.tile([P, E + 1], BF16, tag=f"e{i}")
            nc.scalar.activation(t[:, 0:E], ps[:, 0:E], mybir.ActivationFunctionType.Copy)
            esum = small.tile([P, 1], F32, tag="esum")
            nc.vector.tensor_reduce(esum, t[:, 0:E], axis=mybir.AxisListType.X, op=mybir.AluOpType.add)
            nc.scalar.mul(t[:, E : E + 1], esum, 1.0 / E)
            enct.append(t)

        # kT projection: [dout-chunk, E+1], bf16 matmul (psum accumulates f32)
        for j in range(ND):
            ps = psml.tile([P, E + 1], F32, tag="ps")
            for i in range(ND):
                nc.tensor.matmul(
                    ps,
                    lhsT=wkv[i][:, 128 * j : 128 * (j + 1)],
                    rhs=enct[i][:, 0 : E + 1],
                    start=(i == 0),
                    stop=(i == ND - 1),
                )
            kst = stage.tile([P, E + 1], BF16, tag="kst")
            nc.scalar.activation(kst, ps, mybir.ActivationFunctionType.Copy)
            nc.sync.dma_start(out=kts[bc][j], in_=kst)

        # v projection: natural [t, H*(dh+1)] bf16 with a ones column per head
        # (the AV matmul then emits the softmax denominator as row 64)
        for tch, (toff, tlen) in enumerate(((0, E0), (E0, E1))):
            vst = stage.tile([tlen, H * (DH + 1)], BF16, tag=f"vst{tch}")
            vst3 = vst.rearrange("t (h w) -> t h w", w=DH + 1)
            nc.vector.memset(vst3[:, :, DH : DH + 1], 1.0)
            for half in range(2):
                ps = pbig.tile([tlen, 512], F32, tag="pbig")
                for i in range(ND):
                    nc.tensor.matmul(
                        ps,
                        lhsT=enct[i][:, toff : toff + tlen],
                        rhs=wkv[i][:, D + 512 * half : D + 512 * (half + 1)],
                        start=(i == 0),
                        stop=(i == ND - 1),
                    )
                nc.scalar.activation(
                    vst3[:, 8 * half : 8 * (half + 1), 0:DH],
                    ps.rearrange("t (h w) -> t h w", w=DH),
                    mybir.ActivationFunctionType.Copy,
                )
            nc.sync.dma_start(out=(v0s if tch == 0 else v1s)[bc], in_=vst)

    # ---- load Wq (bf16) into the Wk/Wv slots ----
    wq = []
    for i in range(ND):
        t = wmat.tile([P, D], BF16, tag=f"w{i}")
        nc.gpsimd.dma_start(out=t, in_=d["Wq"][128 * i : 128 * (i + 1), :])
        wq.append(t)

    # ---- main iterations over (b, s-chunk), software-pipelined emission:
    # A(it+1) is emitted between C(it) and D/E(it) so the PE stream has
    # transpose/Q-proj work to chew on while it's coef DMA chain resolves.
    kt = {}
    v0 = {}
    v1 = {}
    ksb = {}

    def emit_kv(b):
        for c in range(C):
            bc = c * B + b
            for j in range(ND):
                t = ktp.tile([P, E + 1], BF16, tag=f"kt{c}_{j}", name=f"kt{c}_{j}")
                nc.sync.dma_start(out=t, in_=kts[bc][j])
                kt[(c, j)] = t
            v0[c] = vp.tile([E0, H * (DH + 1)], BF16, tag=f"v0{c}", name=f"v0{c}")
            nc.sync.dma_start(out=v0[c], in_=v0s[bc])
            v1[c] = vp.tile([E1, H * (DH + 1)], BF16, tag=f"v1{c}", name=f"v1{c}")
            nc.sync.dma_start(out=v1[c], in_=v1s[bc])
        # block-diagonal ksum/E columns for the pooled matmul: per (c, j) a
        # [128, H] tile whose cols 2j, 2j+1 hold kt's column E (zero elsewhere)
        for c in range(C):
            for j in range(ND):
                kb = ksb_p.tile([P, H], BF16, tag=f"ksb{c}_{j}", name=f"ksb{c}_{j}")
                nc.vector.memset(kb, 0.0)
                nc.vector.tensor_copy(kb[0:64, 2 * j : 2 * j + 1], kt[(c, j)][0:64, E : E + 1])
                nc.vector.tensor_copy(kb[64:128, 2 * j + 1 : 2 * j + 2], kt[(c, j)][64:128, E : E + 1])
                ksb[(c, j)] = kb

    def emit_A(b, r0):
        qt = {}
        for c in range(C):
            bc = c * B + b
            xin0 = xin_p.tile([P, D], BF16, tag="xin0")
            xin1 = xin_p.tile([P, D], BF16, tag="xin1")
            nc.gpsimd.dma_start(out=xin0, in_=d["x"][bc, r0 : r0 + 128, :])
            nc.gpsimd.dma_start(out=xin1, in_=d["x"][bc, r0 + 128 : r0 + 256, :])
            xt = []
            for i in range(ND):
                ps = pstp.tile([P, S_TILE], BF16, tag="pst")
                sl = slice(128 * i, 128 * (i + 1))
                nc.tensor.transpose(ps[:, 0:128], xin0[:, sl], ident)
                nc.tensor.transpose(ps[:, 128:256], xin1[:, sl], ident)
                t = xt_p.tile([P, S_TILE], BF16, tag=f"xt{i}", name=f"xt{i}")
                nc.scalar.activation(t, ps, mybir.ActivationFunctionType.Copy)
                xt.append(t)
            for j in range(ND):
                ps = psml.tile([P, S_TILE], F32, tag="ps")
                for i in range(ND):
                    nc.tensor.matmul(
                        ps,
                        lhsT=wq[i][:, 128 * j : 128 * (j + 1)],
                        rhs=xt[i],
                        start=(i == 0),
                        stop=(i == ND - 1),
                    )
                t = qt_p.tile([P, S_TILE], BF16, tag=f"qt{c}_{j}", name=f"qt{c}_{j}")
                nc.scalar.mul(t, ps, SCALE)  # fold dh^-0.5 into q
                qt[(c, j)] = t
        return qt

    def emit_B(qt):
        # pooled/E via M=16 matmuls; matmul base partition must be 0/32/64,
        # so components 0-2 sit in ps_pool at 32c and component 3 in pp2.
        # den_sb collects per-(c,h) softmax denominators at rows 32c+h via
        # sbuf->sbuf DMA (engines cannot write single rows off-alignment).
        ps_pool = psst.tile([P, S_TILE], F32, tag="pp")
        ps_pool2 = psst.tile([H, S_TILE], F32, tag="pp2")
        den_sb = stats.tile([P, S_TILE], F32, tag="densb")

        def pooled_slot(c):
            return (ps_pool[32 * c : 32 * c + H, :]) if c < 3 else (ps_pool2[0:H, :])

        ao = {}
        for c in range(C):
            ao[c] = ao_p.tile([P, ND * S_TILE], BF16, tag="ao", name=f"ao{c}")
            for j in range(ND):
                nc.tensor.matmul(
                    pooled_slot(c),
                    lhsT=ksb[(c, j)],
                    rhs=qt[(c, j)],
                    start=(j == 0),
                    stop=(j == ND - 1),
                )
            for h in range(H):
                j, hr = h // 2, 64 * (h % 2)
                lk = kt[(c, j)]
                rq = qt[(c, j)][hr : hr + 64, :]
                ps_a = psml.tile([P, S_TILE], F32, tag="ps")
                nc.tensor.matmul(ps_a, lhsT=lk[hr : hr + 64, 0:E0], rhs=rq, start=True, stop=True)
                ps_b = psml.tile([E1, S_TILE], F32, tag="ps")
                nc.tensor.matmul(ps_b, lhsT=lk[hr : hr + 64, E0:E], rhs=rq, start=True, stop=True)
                wa = w_p.tile([P, S_TILE], BF16, tag="wa")
                nc.scalar.activation(wa, ps_a, mybir.ActivationFunctionType.Exp)
                wb = w_p.tile([E1, S_TILE], BF16, tag="wb")
                nc.scalar.activation(wb, ps_b, mybir.ActivationFunctionType.Exp)
                # attention-value product (unnormalized); row 64 = denom
                ps_av = psml.tile([DH + 1, S_TILE], F32, tag="ps")
                v_sl = slice((DH + 1) * h, (DH + 1) * (h + 1))
                nc.tensor.matmul(ps_av, lhsT=v0[c][:, v_sl], rhs=wa, start=True, stop=False)
                nc.tensor.matmul(ps_av, lhsT=v1[c][:, v_sl], rhs=wb, start=False, stop=True)
                nc.vector.tensor_copy(ao[c][hr : hr + 64, S_TILE * j : S_TILE * (j + 1)], ps_av[0:DH, :])
                den_st = denst_p.tile([1, S_TILE], F32, tag="denst")
                nc.vector.tensor_copy(den_st, ps_av[DH : DH + 1, :])
                nc.sync.dma_start(out=den_sb[32 * c + h : 32 * c + h + 1, :], in_=den_st)
        return ao, pooled_slot, den_sb

    def emit_C(pooled_slot, den_sb):
        # coef = softmax_c(exp(pooled)) / denom.  TensorTensor ops need
        # identical partition ranges on HW, so per-component stats live in
        # [16, S] tiles at offset 0.
        ep = [stats.tile([H, S_TILE], F32, tag=f"ep{c}", name=f"ep{c}") for c in range(C)]
        rd = [stats.tile([H, S_TILE], F32, tag=f"rd{c}", name=f"rd{c}") for c in range(C)]
        coefc = [stats.tile([H, S_TILE], F32, tag=f"coefc{c}", name=f"coefc{c}") for c in range(C)]
        for c in range(C):
            nc.scalar.activation(ep[c], pooled_slot(c), mybir.ActivationFunctionType.Exp)
            nc.vector.reciprocal(rd[c], den_sb[32 * c : 32 * c + H, :])
        sc_sum = stats.tile([H, S_TILE], F32, tag="sc")
        nc.vector.tensor_add(sc_sum, ep[0], ep[1])
        nc.vector.tensor_add(sc_sum, sc_sum, ep[2])
        nc.vector.tensor_add(sc_sum, sc_sum, ep[3])
        rs = stats.tile([H, S_TILE], F32, tag="rs")
        nc.vector.reciprocal(rs, sc_sum)
        for c in range(C):
            nc.vector.tensor_mul(coefc[c], ep[c], rs)
            nc.vector.tensor_mul(coefc[c], coefc[c], rd[c])
        coef_d = dram.tile([C * H, S_TILE], F32, tag="coefd", name="coef_d", bufs=2)
        for c in range(C):
            nc.sync.dma_start(out=coef_d[H * c : H * (c + 1), :], in_=coefc[c])
        return coef_d

    def emit_D(ao, coef_d):
        # scale aoT in place by coef (broadcast over dh via a DRAM bounce:
        # SBUF sources cannot have stride-0 partitions, DRAM can).  One
        # [128, S] broadcast per (c, head-pair): rows 0:64 get head 2j,
        # rows 64:128 get head 2j+1, matching the ao tile layout.
        for c in range(C):
            for j in range(ND):
                cb = coefb_p.tile([P, S_TILE], F32, tag="cb")
                src2 = coef_d[c * H + 2 * j : c * H + 2 * j + 2, :]
                src2 = bass.AP(
                    tensor=src2.tensor,
                    offset=src2.offset,
                    ap=[list(src2.ap[0]), [0, 64]] + [list(a) for a in src2.ap[1:]],
                )
                nc.sync.dma_start(out=cb, in_=src2)
                sl_ao = ao[c][:, S_TILE * j : S_TILE * (j + 1)]
                nc.vector.tensor_mul(sl_ao, sl_ao, cb)

    def emit_E(b, r0, ao):
        for c in range(C):
            bc = c * B + b
            for m in range(2):
                rows = slice(r0 + 128 * m, r0 + 128 * (m + 1))
                for half in range(2):
                    cols = slice(512 * half, 512 * (half + 1))
                    ps = pbig.tile([P, 512], F32, tag="pbig")
                    nc.tensor.matmul(ps, lhsT=ones_row, rhs=bo_bf[:, cols], start=True, stop=False)
                    for i in range(ND):
                        nc.tensor.matmul(
                            ps,
                            lhsT=ao[c][:, S_TILE * i + 128 * m : S_TILE * i + 128 * (m + 1)],
                            rhs=wo[i][:, cols],
                            start=False,
                            stop=(i == ND - 1),
                        )
                    xr = xr_p.tile([P, 512], F32, tag="xr")
                    nc.sync.dma_start(out=xr, in_=d["x"][bc, rows, cols])
                    oh = oh_p.tile([P, 512], F32, tag="oh")
                    nc.vector.tensor_add(oh, ps, xr)
                    nc.sync.dma_start(out=d["out"][bc, rows, cols], in_=oh)

    iters = [(b, sc) for b in range(B) for sc in range(n_sc)]
    pend = None  # (b, r0, ao, coef_d) awaiting D/E
    for b, sc in iters:
        if sc == 0:
            emit_kv(b)
        r0 = sc * S_TILE
        qt = emit_A(b, r0)
        if pend is not None:
            emit_D(pend[2], pend[3])
            emit_E(pend[0], pend[1], pend[2])
        ao, pooled_slot, den_sb = emit_B(qt)
        coef_d = emit_C(pooled_slot, den_sb)
        pend = (b, r0, ao, coef_d)
    emit_D(pend[2], pend[3])
    emit_E(pend[0], pend[1], pend[2])


def build_program(s_loc=S_LOC, n_cores=N_CORES):
    nc = bacc.Bacc(trn_type="TRN2", target_bir_lowering=False, debug=False, num_devices=n_cores)
    d = {
        "x": nc.dram_tensor("x", [BC, s_loc, D], F32, kind="ExternalInput").ap(),
        "enc": nc.dram_tensor("enc", [BC, E, D], F32, kind="ExternalInput").ap(),
        "Wq": nc.dram_tensor("Wq", [D, D], F32, kind="ExternalInput").ap(),
        "Wk": nc.dram_tensor("Wk", [D, D], F32, kind="ExternalInput").ap(),
        "Wv": nc.dram_tensor("Wv", [D, D], F32, kind="ExternalInput").ap(),
        "Wo": nc.dram_tensor("Wo", [D, D], F32, kind="ExternalInput").ap(),
        "bo": nc.dram_tensor("bo", [1, D], F32, kind="ExternalInput").ap(),
        "out": nc.dram_tensor("out", [BC, s_loc, D], F32, kind="ExternalOutput").ap(),
    }
    with TileContext(nc, trace_sim=False) as tc, ExitStack() as ctx:
        build_body(ctx, tc, d, s_loc)
    nc.compile()
    return nc


def make_in_maps(hidden_states, encoder_hidden_states, Wq, Wk, Wv, Wo, bo, s_loc=S_LOC, n_cores=N_CORES):
    common = {
        "enc": np.ascontiguousarray(encoder_hidden_states, dtype=np.float32),
        "Wq": np.ascontiguousarray(Wq, dtype=np.float32),
        "Wk": np.ascontiguousarray(Wk, dtype=np.float32),
        "Wv": np.ascontiguousarray(Wv, dtype=np.float32),
        "Wo": np.ascontiguousarray(Wo, dtype=np.float32),
        "bo": np.ascontiguousarray(bo, dtype=np.float32).reshape(1, D),
    }
    return [
        {"x": np.ascontiguousarray(hidden_states[:, i * s_loc : (i + 1) * s_loc, :], dtype=np.float32), **common}
        for i in range(n_cores)
    ]


_NC = None


def kernel(hidden_states, encoder_hidden_states, Wq, Wk, Wv, Wo, bo):
    global _NC
    if _NC is None:
        _NC = build_program()
    in_maps = make_in_maps(hidden_states, encoder_hidden_states, Wq, Wk, Wv, Wo, bo)
    res = run_bass_kernel_spmd(_NC, in_maps, list(range(N_CORES))).results
    out = np.concatenate([res[i]["out"] for i in range(N_CORES)], axis=1)
    return np.ascontiguousarray(out, dtype=np.float32)


if __name__ == "__main__":
    rng = np.random.default_rng(0)
    ins = {
        "hidden_states": rng.standard_normal((BC, S, D), dtype=np.float32),
        "encoder_hidden_states": rng.standard_normal((BC, E, D), dtype=np.float32),
        "Wq": rng.standard_normal((D, D), dtype=np.float32) * 0.02,
        "Wk": rng.standard_normal((D, D), dtype=np.float32) * 0.02,
        "Wv": rng.standard_normal((D, D), dtype=np.float32) * 0.02,
        "Wo": rng.standard_normal((D, D), dtype=np.float32) * 0.02,
        "bo": np.zeros((D,), np.float32),
    }
    out = kernel(**ins)
    print("out", out.shape, out.dtype, float(np.abs(out).max()))



# revision 10
# speedup vs baseline: 1.5374x; 1.5374x over previous
"""Trainium2 Bass kernel for DecomposingAttnProcessor (pooled component softmax
cross-attention), sharded over 8 NeuronCores along the latent-token axis S.

Math (per batch-component bc = c*B + b):
    q = x @ Wq ; k = enc @ Wk ; v = enc @ Wv           (per-head, dh = 64)
    scores = (q k^T) * dh^-0.5                          [H, S, E]
    pooled = mean_E scores ; wp = softmax_c(pooled)
    w = softmax_E(scores) * wp
    out = (w v) @ Wo + bo + x

V2 design (per core, S_loc = 512 rows of S for all 8 bc):
  - dh^-0.5 folded into kT at the encoder stage; kT/v/ksb for all 8 bc stay
    resident in SBUF (no DRAM spill).
  - scoresT[e, s] per head; E1 parts of 4 heads packed in one PSUM bank at
    bases 0/32/64/96 -> one exp per quad.  AV head-pairs share a bank at
    bases 0/64 -> one [128, 512] eviction per pair.
  - softmax denominators emitted as a stacked [16, 512] PSUM block per
    component via zero-padded ones-column matmuls (lhsT = Z[:, h:16], col 15
    ones): den of head h lands on partition 15-h, accumulating zeros above.
    No row-copies, no row-DMAs.
  - pooled rows stacked per component at base 32c via ksb block-diag
    matmuls (ksum columns built by DVE reduces at the encoder stage); the
    mean's 1/E is folded into the exp eviction's ACT scale.
  - coef = softmax_c(exp(pooled))/den on [16, 512] tiles; broadcast across
    partitions via a PE selector matmul into PSUM; DVE multiplies ao in
    place with in2 = PSUM (no DRAM bounce).
  - Head/pooled/den stacks all use reversed row order (partition 15-h).
  - big batched DMAs only: ~45 per iteration vs ~130 in the baseline.
"""

import sys
from contextlib import ExitStack

sys.path.insert(0, "/opt/trn_rl_repo")

import numpy as np

import concourse.bass as bass  # noqa: E402
from concourse import bacc, mybir  # noqa: E402
from concourse.bass_utils import run_bass_kernel_spmd  # noqa: E402
from concourse.masks import make_identity  # noqa: E402
from concourse.tile import TileContext  # noqa: E402

# Problem dims (hardcoded per spec)
BC, S, D, E, H, C = 8, 4096, 1024, 160, 16, 4
B = BC // C  # 2
DH = D // H  # 64
SCALE = DH**-0.5  # 0.125
N_CORES = 8
S_LOC = S // N_CORES  # 512 rows of S per core
E0, E1 = 128, E - 128  # encoder-token chunks (128 + 32)
ND = D // 128  # 8 chunks of the hidden dim

F32 = mybir.dt.float32
BF16 = mybir.dt.bfloat16
EXP = mybir.ActivationFunctionType.Exp
COPY = mybir.ActivationFunctionType.Copy


def build_body(ctx, tc, d, s_loc):
    nc = tc.nc
    ctx.enter_context(
        nc.allow_low_precision(reason="bf16 stats are within the 2e-2 rel-err budget")
    )
    P = 128
    SL = s_loc  # 512

    pools = {}

    def pool(name, bufs, space="SBUF"):
        if name not in pools:
            pools[name] = ctx.enter_context(tc.tile_pool(name=name, bufs=bufs, space=space))
        return pools[name]

    const = pool("const", 1)
    wres = pool("wres", 1)    # Wq / Wo resident bf16
    kv_p = pool("kv", 1)      # kt / v0 / v1 / ksb resident for all 8 bc
    enc_p = pool("enc", 2)
    enct_p = pool("enct", 1)
    xin_p = pool("xin", 2)
    xt_p = pool("xt", 1)
    qt_p = pool("qt", 2)
    wa_p = pool("wa", 1)
    wb_p = pool("wb", 2)
    ao_p = pool("ao", 4)      # also hosts Wk/Wv during the encoder phase
    st_p = pool("st", 1)
    xr_p = pool("xr", 2)
    oh_p = pool("oh", 1)

    # PSUM: exactly 8 banks
    psA = pool("psA", 2, space="PSUM")    # E0 scores / kT-proj
    psEAV = pool("psEAV", 2, space="PSUM")  # E1 quads + AV pairs / v-proj
    psO = pool("psO", 2, space="PSUM")    # xT transposes / Q-proj / cb / O-proj
    psPL = pool("psPL", 1, space="PSUM")  # pooled stack [4c x 16, 512]
    psDN = pool("psDN", 1, space="PSUM")  # denominator stack [4c x 16, 512]

    # ---- constants ----
    ident = const.tile([P, P], BF16, tag="ident")
    make_identity(nc, ident)
    ones1 = const.tile([1, P], BF16, tag="ones1")
    nc.vector.memset(ones1, 1.0)
    bo_bf = const.tile([1, D], BF16, tag="bo_bf")
    nc.gpsimd.dma_start(out=bo_bf, in_=d["bo"])  # f32 -> bf16 cast DMA
    # Z: ones at col 15 only; lhsT = Z[rows, h:16] puts a ones-column at out
    # partition 15-h with zeros accumulated above it (den stacks).
    zden = const.tile([P, 16], BF16, tag="zden")
    nc.vector.memset(zden, 0.0)
    nc.vector.memset(zden[:, 15:16], 1.0)
    # sel[j]: [16, 128] selector: cb[p, s] = coef[15 - (2j + (p>=64)), s].
    # Built via PE transpose because engines cannot write single rows at
    # unaligned partitions.
    sel = []
    for j in range(ND):
        selt = const.tile([P, 16], BF16, tag=f"selt{j}", name=f"selt{j}")
        nc.vector.memset(selt, 0.0)
        nc.vector.memset(selt[0:64, 15 - 2 * j : 16 - 2 * j], 1.0)
        nc.vector.memset(selt[64:128, 14 - 2 * j : 15 - 2 * j], 1.0)
        pss = psO.tile([P, 512], F32, tag="ps", name="pss").bitcast(BF16)
        nc.tensor.transpose(pss[0:16, 0:P], selt, ident)
        t = const.tile([16, P], BF16, tag=f"sel{j}", name=f"sel{j}")
        nc.scalar.activation(t, pss[0:16, 0:P], COPY)
        sel.append(t)

    # ---- weights (batched bf16 cast DMAs) ----
    wq = wres.tile([P, ND * D], BF16, tag="wq")
    wo = wres.tile([P, ND * D], BF16, tag="wo")
    nc.gpsimd.dma_start(out=wq, in_=d["Wq"].rearrange("(n p) d -> p n d", p=P))
    nc.gpsimd.dma_start(out=wo, in_=d["Wo"].rearrange("(n p) d -> p n d", p=P))
    # Wk/Wv live in ao-tagged tiles (dead after the encoder phase)
    wk_lo = ao_p.tile([P, ND * SL], BF16, tag="ao", name="wk_lo")
    wk_hi = ao_p.tile([P, ND * SL], BF16, tag="ao", name="wk_hi")
    wv_lo = ao_p.tile([P, ND * SL], BF16, tag="ao", name="wv_lo")
    wv_hi = ao_p.tile([P, ND * SL], BF16, tag="ao", name="wv_hi")
    nc.gpsimd.dma_start(out=wk_lo, in_=d["Wk"].rearrange("(n p) d -> p n d", p=P)[:, 0:4, :])
    nc.gpsimd.dma_start(out=wk_hi, in_=d["Wk"].rearrange("(n p) d -> p n d", p=P)[:, 4:8, :])
    nc.gpsimd.dma_start(out=wv_lo, in_=d["Wv"].rearrange("(n p) d -> p n d", p=P)[:, 0:4, :])
    nc.gpsimd.dma_start(out=wv_hi, in_=d["Wv"].rearrange("(n p) d -> p n d", p=P)[:, 4:8, :])

    def wslice(lo, hi, i, c0, c1):
        t = lo if i < 4 else hi
        return t[:, D * (i % 4) + c0 : D * (i % 4) + c1]

    # ---- encoder phase: kT (scaled) + ksb + v0/v1 for all 8 bc ----
    kt, v0, v1, ksb = {}, {}, {}, {}
    for bc in range(BC):
        en0 = enc_p.tile([P, D], BF16, tag="en0")
        en1 = enc_p.tile([E1, D], BF16, tag="en1")
        nc.gpsimd.dma_start(out=en0, in_=d["enc"][bc, 0:E0, :])
        nc.gpsimd.dma_start(out=en1, in_=d["enc"][bc, E0:E, :])

        enct = []
        for i in range(ND):
            pst = psO.tile([P, 512], F32, tag="ps", name="pst").bitcast(BF16)[:, 0:E]
            sl = slice(128 * i, 128 * (i + 1))
            nc.tensor.transpose(pst[:, 0:E0], en0[:, sl], ident)
            nc.tensor.transpose(pst[:, E0:E], en1[:, sl], ident[0:E1, 0:E1])
            t = enct_p.tile([P, E], BF16, tag=f"e{i}", name=f"e{i}_{bc}")
            nc.scalar.activation(t, pst, COPY)
            enct.append(t)

        ktile = kv_p.tile([P, ND * E], BF16, tag=f"kt{bc}", name=f"kt{bc}")
        kt[bc] = ktile
        for j in range(ND):
            ps = psA.tile([P, 512], F32, tag="ps")
            for i in range(ND):
                nc.tensor.matmul(
                    ps[:, 0:E],
                    lhsT=wslice(wk_lo, wk_hi, i, 128 * j, 128 * (j + 1)),
                    rhs=enct[i],
                    start=(i == 0),
                    stop=(i == ND - 1),
                )
            ksl = ktile[:, E * j : E * (j + 1)]
            nc.scalar.activation(ksl, ps[:, 0:E], COPY, scale=SCALE)
            kb = kv_p.tile([P, 16], BF16, tag=f"ksb{bc}_{j}", name=f"ksb{bc}_{j}")
            nc.gpsimd.memset(kb, 0.0)
            # head 2j ksum -> col 15-2j (rows 0:64); head 2j+1 -> col 14-2j
            nc.vector.tensor_reduce(
                kb[0:64, 15 - 2 * j : 16 - 2 * j], ksl[0:64, :],
                axis=mybir.AxisListType.X, op=mybir.AluOpType.add,
            )
            nc.vector.tensor_reduce(
                kb[64:128, 14 - 2 * j : 15 - 2 * j], ksl[64:128, :],
                axis=mybir.AxisListType.X, op=mybir.AluOpType.add,
            )
            ksb[(bc, j)] = kb

        v0t = kv_p.tile([P, D], BF16, tag=f"v0_{bc}", name=f"v0_{bc}")
        v1t = kv_p.tile([P, D], BF16, tag=f"v1_{bc}", name=f"v1_{bc}")
        v0[bc], v1[bc] = v0t, v1t
        for half in range(2):
            cols = slice(512 * half, 512 * (half + 1))
            ps0 = psEAV.tile([P, 512], F32, tag="ps")
            ps1 = psEAV.tile([P, 512], F32, tag="ps")
            for i in range(ND):
                nc.tensor.matmul(
                    ps0, lhsT=enct[i][:, 0:E0], rhs=wslice(wv_lo, wv_hi, i, 512 * half, 512 * (half + 1)),
                    start=(i == 0), stop=(i == ND - 1),
                )
            for i in range(ND):
                nc.tensor.matmul(
                    ps1[0:E1, :], lhsT=enct[i][:, E0:E], rhs=wslice(wv_lo, wv_hi, i, 512 * half, 512 * (half + 1)),
                    start=(i == 0), stop=(i == ND - 1),
                )
            nc.scalar.activation(v0t[:, cols], ps0, COPY)
            # replicate v1 rows at partition bases 0 and 64
            nc.scalar.activation(v1t[0:E1, cols], ps1[0:E1, :], COPY)
            nc.vector.tensor_copy(v1t[64 : 64 + E1, cols], ps1[0:E1, :])

    # ---- main loop ----
    def emit_A(b, c):
        """x load + transpose + Q-projection for one component."""
        bc = c * B + b
        xin = xin_p.tile([P, 4 * D], BF16, tag="xin")
        nc.gpsimd.dma_start(out=xin, in_=d["x"][bc].rearrange("(k p) d -> p k d", p=P))
        xt = []
        for i in range(ND):
            pst = psO.tile([P, 512], F32, tag="ps", name="pst").bitcast(BF16)[:, 0:SL]
            for k in range(4):
                nc.tensor.transpose(
                    pst[:, 128 * k : 128 * (k + 1)],
                    xin[:, D * k + 128 * i : D * k + 128 * (i + 1)],
                    ident,
                )
            t = xt_p.tile([P, SL], BF16, tag=f"xt{i}", name=f"xt{i}_{bc}")
            nc.scalar.activation(t, pst, COPY)
            xt.append(t)
        qt = qt_p.tile([P, ND * SL], BF16, tag="qt")
        for j in range(ND):
            ps = psO.tile([P, 512], F32, tag="ps")
            for i in range(ND):
                nc.tensor.matmul(
                    ps,
                    lhsT=wq[:, D * i + 128 * j : D * i + 128 * (j + 1)],
                    rhs=xt[i],
                    start=(i == 0),
                    stop=(i == ND - 1),
                )
            nc.scalar.activation(qt[:, SL * j : SL * (j + 1)], ps, COPY)
        return qt

    def emit_B(b, c, qt, pl_ps, dn_ps, ao):
        """Scores + exp + AV + pooled-mm + den-mm for one component.

        Pooled/den stacks live at partition block 32c (rows 32c .. 32c+16);
        head h of the component sits at partition 32c + 15-h.  Block 96 needs
        explicit tile_position (the inferred path rejects base 96).
        """
        bc = c * B + b
        ktile = kt[bc]
        blk = 32 * c
        for j in range(ND):
            nc.tensor.matmul(
                pl_ps[blk : blk + 16, :],
                lhsT=ksb[(bc, j)],
                rhs=qt[:, SL * j : SL * (j + 1)],
                start=(j == 0),
                stop=(j == ND - 1),
                skip_group_check=True,
                tile_position=(0, blk),
            )
        for j in range(ND):  # head pairs (2j, 2j+1)
            psb = psEAV.tile([P, 512], F32, tag="ps", name="psb")
            was = []
            for hp in range(2):
                h = 2 * j + hp
                hr = 64 * hp
                qsl = qt[hr : hr + 64, SL * j : SL * (j + 1)]
                ps_a = psA.tile([P, 512], F32, tag="ps", name="ps_a")
                nc.tensor.matmul(
                    ps_a, lhsT=ktile[hr : hr + 64, E * j : E * j + E0], rhs=qsl,
                    start=True, stop=True,
                )
                nc.tensor.matmul(
                    psb[64 * hp : 64 * hp + E1, :],
                    lhsT=ktile[hr : hr + 64, E * j + E0 : E * j + E],
                    rhs=qsl,
                    start=True, stop=True, skip_group_check=True,
                )
                wa = wa_p.tile([P, SL], BF16, tag=f"wa{hp}", name=f"wa{hp}")
                nc.scalar.activation(wa, ps_a, EXP)
                was.append(wa)
            wb = wb_p.tile([P, SL], BF16, tag="wb", name="wb")
            nc.scalar.activation(wb[0 : 64 + E1, :], psb[0 : 64 + E1, :], EXP)
            ps_av = psEAV.tile([P, 512], F32, tag="ps", name="ps_av")
            for hp in range(2):
                h = 2 * j + hp
                hr = 64 * hp
                wa = was[hp]
                wbs = wb[64 * hp : 64 * hp + E1, :]
                vsl = slice(64 * h, 64 * (h + 1))
                nc.tensor.matmul(
                    ps_av[hr : hr + 64, :], lhsT=v0[bc][:, vsl], rhs=wa,
                    start=True, stop=False, skip_group_check=True,
                )
                nc.tensor.matmul(
                    ps_av[hr : hr + 64, :],
                    lhsT=v1[bc][64 * hp : 64 * hp + E1, vsl],
                    rhs=wbs,
                    start=False, stop=True, skip_group_check=True,
                )
                # denominator stack: den_h -> partition 32c + 15-h
                nc.tensor.matmul(
                    dn_ps[blk : blk + 16 - h, :],
                    lhsT=zden[:, h:16], rhs=wa,
                    start=(h == 0), stop=False,
                    skip_group_check=True,
                    tile_position=(0, blk),
                )
                nc.tensor.matmul(
                    dn_ps[blk : blk + 16 - h, :],
                    lhsT=zden[64 * hp : 64 * hp + E1, h:16],
                    rhs=wbs,
                    start=False, stop=(h == H - 1),
                    skip_group_check=True,
                    tile_position=(64 * hp, blk),
                )
            nc.vector.tensor_copy(ao[:, SL * j : SL * (j + 1)], ps_av)

    def emit_C(pl_ps, dn_ps):
        """coef[c] = softmax_c(exp(pooled/E)) / den, rows in 15-h order."""
        ep = [st_p.tile([16, SL], BF16, tag=f"ep{c}", name=f"ep{c}") for c in range(C)]
        rd = [st_p.tile([16, SL], BF16, tag=f"rd{c}", name=f"rd{c}") for c in range(C)]
        for c in range(C):
            nc.scalar.activation(ep[c], pl_ps[32 * c : 32 * c + 16, :], EXP, scale=1.0 / E)
            nc.vector.reciprocal(rd[c], dn_ps[32 * c : 32 * c + 16, :])
        sc = st_p.tile([16, SL], BF16, tag="sc", name="sc")
        nc.vector.tensor_add(sc, ep[0], ep[1])
        nc.vector.tensor_add(sc, sc, ep[2])
        nc.vector.tensor_add(sc, sc, ep[3])
        rs = st_p.tile([16, SL], BF16, tag="rs", name="rs")
        nc.vector.reciprocal(rs, sc)
        coefs = []
        for c in range(C):
            cf = st_p.tile([16, SL], BF16, tag=f"cf{c}", name=f"cf{c}")
            nc.vector.tensor_mul(rd[c], rd[c], rs)
            nc.vector.tensor_mul(cf, ep[c], rd[c])
            coefs.append(cf)
        return coefs

    def emit_D(aos, coefs):
        """ao *= broadcast(coef): PE selector matmul + DVE mul (in2 = PSUM)."""
        for c in range(C):
            for j in range(ND):
                cb = psO.tile([P, 512], F32, tag="ps", name="cb")
                nc.tensor.matmul(cb, lhsT=sel[j], rhs=coefs[c], start=True, stop=True)
                sl_ao = aos[c][:, SL * j : SL * (j + 1)]
                nc.vector.tensor_mul(sl_ao, sl_ao, cb)

    def emit_E(b, aos):
        """O-projection + bias + residual + store."""
        for c in range(C):
            bc = c * B + b
            for m in range(4):
                xr = xr_p.tile([P, D], F32, tag="xr")
                nc.sync.dma_start(out=xr, in_=d["x"][bc, 128 * m : 128 * (m + 1), :])
                oh = oh_p.tile([P, D], F32, tag="oh")
                for half in range(2):
                    cols = slice(512 * half, 512 * (half + 1))
                    ps = psO.tile([P, 512], F32, tag="ps")
                    nc.tensor.matmul(ps, lhsT=ones1, rhs=bo_bf[:, cols], start=True, stop=False)
                    for i in range(ND):
                        nc.tensor.matmul(
                            ps,
                            lhsT=aos[c][:, SL * i + 128 * m : SL * i + 128 * (m + 1)],
                            rhs=wo[:, D * i + 512 * half : D * i + 512 * (half + 1)],
                            start=False,
                            stop=(i == ND - 1),
                        )
                    nc.vector.tensor_add(oh[:, cols], ps, xr[:, cols])
                nc.sync.dma_start(out=d["out"][bc, 128 * m : 128 * (m + 1), :], in_=oh)

    pend = None
    for b in range(B):
        pl_ps = psPL.tile([P, 512], F32, tag="ps", name=f"pl{b}")
        dn_ps = psDN.tile([P, 512], F32, tag="ps", name=f"dn{b}")
        aos = {}
        for c in range(C):
            qt = emit_A(b, c)
            if c == 1 and pend is not None:
                emit_D(pend[1], pend[2])
                emit_E(pend[0], pend[1])
                pend = None
            aos[c] = ao_p.tile([P, ND * SL], BF16, tag="ao", name=f"ao{c}_{b}")
            emit_B(b, c, qt, pl_ps, dn_ps, aos[c])
        coefs = emit_C(pl_ps, dn_ps)
        pend = (b, aos, coefs)
    emit_D(pend[1], pend[2])
    emit_E(pend[0], pend[1])


def build_program(s_loc=S_LOC, n_cores=N_CORES):
    nc = bacc.Bacc(trn_type="TRN2", target_bir_lowering=False, debug=False, num_devices=n_cores)
    d = {
        "x": nc.dram_tensor("x", [BC, s_loc, D], F32, kind="ExternalInput").ap(),
        "enc": nc.dram_tensor("enc", [BC, E, D], F32, kind="ExternalInput").ap(),
        "Wq": nc.dram_tensor("Wq", [D, D], F32, kind="ExternalInput").ap(),
        "Wk": nc.dram_tensor("Wk", [D, D], F32, kind="ExternalInput").ap(),
        "Wv": nc.dram_tensor("Wv", [D, D], F32, kind="ExternalInput").ap(),
        "Wo": nc.dram_tensor("Wo", [D, D], F32, kind="ExternalInput").ap(),
        "bo": nc.dram_tensor("bo", [1, D], F32, kind="ExternalInput").ap(),
        "out": nc.dram_tensor("out", [BC, s_loc, D], F32, kind="ExternalOutput").ap(),
    }
    with TileContext(nc, trace_sim=False) as tc, ExitStack() as ctx:
        build_body(ctx, tc, d, s_loc)
    nc.compile()
    return nc


def make_in_maps(hidden_states, encoder_hidden_states, Wq, Wk, Wv, Wo, bo, s_loc=S_LOC, n_cores=N_CORES):
    common = {
        "enc": np.ascontiguousarray(encoder_hidden_states, dtype=np.float32),
        "Wq": np.ascontiguousarray(Wq, dtype=np.float32),
        "Wk": np.ascontiguousarray(Wk, dtype=np.float32),
        "Wv": np.ascontiguousarray(Wv, dtype=np.float32),
        "Wo": np.ascontiguousarray(Wo, dtype=np.float32),
        "bo": np.ascontiguousarray(bo, dtype=np.float32).reshape(1, D),
    }
    return [
        {"x": np.ascontiguousarray(hidden_states[:, i * s_loc : (i + 1) * s_loc, :], dtype=np.float32), **common}
        for i in range(n_cores)
    ]


_NC = None


def kernel(hidden_states, encoder_hidden_states, Wq, Wk, Wv, Wo, bo):
    global _NC
    if _NC is None:
        _NC = build_program()
    in_maps = make_in_maps(hidden_states, encoder_hidden_states, Wq, Wk, Wv, Wo, bo)
    res = run_bass_kernel_spmd(_NC, in_maps, list(range(N_CORES))).results
    out = np.concatenate([res[i]["out"] for i in range(N_CORES)], axis=1)
    return np.ascontiguousarray(out, dtype=np.float32)


if __name__ == "__main__":
    build_program()
    print("compile OK")


# revision 14
# speedup vs baseline: 1.5834x; 1.0299x over previous
"""Trainium2 Bass kernel for DecomposingAttnProcessor (pooled component softmax
cross-attention), sharded over 8 NeuronCores along the latent-token axis S.

Math (per batch-component bc = c*B + b):
    q = x @ Wq ; k = enc @ Wk ; v = enc @ Wv           (per-head, dh = 64)
    scores = (q k^T) * dh^-0.5                          [H, S, E]
    pooled = mean_E scores ; wp = softmax_c(pooled)
    w = softmax_E(scores) * wp
    out = (w v) @ Wo + bo + x

V2 design (per core, S_loc = 512 rows of S for all 8 bc):
  - dh^-0.5 folded into kT at the encoder stage; kT/v/ksb for all 8 bc stay
    resident in SBUF (no DRAM spill).
  - scoresT[e, s] per head; E1 parts of 4 heads packed in one PSUM bank at
    bases 0/32/64/96 -> one exp per quad.  AV head-pairs share a bank at
    bases 0/64 -> one [128, 512] eviction per pair.
  - softmax denominators emitted as a stacked [16, 512] PSUM block per
    component via zero-padded ones-column matmuls (lhsT = Z[:, h:16], col 15
    ones): den of head h lands on partition 15-h, accumulating zeros above.
    No row-copies, no row-DMAs.
  - pooled rows stacked per component at base 32c via ksb block-diag
    matmuls (ksum columns built by DVE reduces at the encoder stage); the
    mean's 1/E is folded into the exp eviction's ACT scale.
  - coef = softmax_c(exp(pooled))/den on [16, 512] tiles; broadcast across
    partitions via a PE selector matmul into PSUM; DVE multiplies ao in
    place with in2 = PSUM (no DRAM bounce).
  - Head/pooled/den stacks all use reversed row order (partition 15-h).
  - big batched DMAs only: ~45 per iteration vs ~130 in the baseline.
"""

import sys
from contextlib import ExitStack

sys.path.insert(0, "/opt/trn_rl_repo")

import numpy as np

import concourse.bass as bass  # noqa: E402
from concourse import bacc, mybir  # noqa: E402
from concourse.bass_utils import run_bass_kernel_spmd  # noqa: E402
from concourse.masks import make_identity  # noqa: E402
from concourse.tile import TileContext  # noqa: E402

# Problem dims (hardcoded per spec)
BC, S, D, E, H, C = 8, 4096, 1024, 160, 16, 4
B = BC // C  # 2
DH = D // H  # 64
SCALE = DH**-0.5  # 0.125
N_CORES = 8
S_LOC = S // N_CORES  # 512 rows of S per core
E0, E1 = 128, E - 128  # encoder-token chunks (128 + 32)
ND = D // 128  # 8 chunks of the hidden dim

F32 = mybir.dt.float32
BF16 = mybir.dt.bfloat16
EXP = mybir.ActivationFunctionType.Exp
COPY = mybir.ActivationFunctionType.Copy


def build_body(ctx, tc, d, s_loc):
    nc = tc.nc
    ctx.enter_context(
        nc.allow_low_precision(reason="bf16 stats are within the 2e-2 rel-err budget")
    )
    P = 128
    SL = s_loc  # 512

    pools = {}

    def pool(name, bufs, space="SBUF"):
        if name not in pools:
            pools[name] = ctx.enter_context(tc.tile_pool(name=name, bufs=bufs, space=space))
        return pools[name]

    const = pool("const", 1)
    wres = pool("wres", 1)    # Wq / Wo resident bf16
    kv_p = pool("kv", 1)      # kt / v0 / v1 / ksb resident for all 8 bc
    enc_p = pool("enc", 2)
    enct_p = pool("enct", 1)
    xin_p = pool("xin", 2)
    xt_p = pool("xt", 1)
    qt_p = pool("qt", 2)
    wa_p = pool("wa", 1)
    wb_p = pool("wb", 2)
    ao_p = pool("ao", 4)      # also hosts Wk/Wv during the encoder phase
    st_p = pool("st", 1)
    xr_p = pool("xr", 2)
    oh_p = pool("oh", 1)

    # PSUM: exactly 8 banks
    psA = pool("psA", 2, space="PSUM")    # E0 scores / kT-proj
    psEAV = pool("psEAV", 2, space="PSUM")  # E1 quads + AV pairs / v-proj
    psO = pool("psO", 2, space="PSUM")    # xT transposes / Q-proj / cb / O-proj
    psPL = pool("psPL", 1, space="PSUM")  # pooled stack [4c x 16, 512]
    psDN = pool("psDN", 1, space="PSUM")  # denominator stack [4c x 16, 512]

    # ---- constants ----
    ident = const.tile([P, P], BF16, tag="ident")
    make_identity(nc, ident)
    ones1 = const.tile([1, P], BF16, tag="ones1")
    nc.vector.memset(ones1, 1.0)
    bo_bf = const.tile([1, D], BF16, tag="bo_bf")
    nc.gpsimd.dma_start(out=bo_bf, in_=d["bo"])  # f32 -> bf16 cast DMA
    # Z: ones at col 15 only; lhsT = Z[rows, h:16] puts a ones-column at out
    # partition 15-h with zeros accumulated above it (den stacks).
    zden = const.tile([P, 16], BF16, tag="zden")
    nc.vector.memset(zden, 0.0)
    nc.vector.memset(zden[:, 15:16], 1.0)
    # sel[j]: [16, 128] selector: cb[p, s] = coef[15 - (2j + (p>=64)), s].
    # Built via PE transpose because engines cannot write single rows at
    # unaligned partitions.
    sel = []
    for j in range(ND):
        selt = const.tile([P, 16], BF16, tag=f"selt{j}", name=f"selt{j}")
        nc.vector.memset(selt, 0.0)
        nc.vector.memset(selt[0:64, 15 - 2 * j : 16 - 2 * j], 1.0)
        nc.vector.memset(selt[64:128, 14 - 2 * j : 15 - 2 * j], 1.0)
        pss = psO.tile([P, 512], F32, tag="ps", name="pss").bitcast(BF16)
        nc.tensor.transpose(pss[0:16, 0:P], selt, ident)
        t = const.tile([16, P], BF16, tag=f"sel{j}", name=f"sel{j}")
        nc.scalar.activation(t, pss[0:16, 0:P], COPY)
        sel.append(t)

    # ---- weights (batched bf16 cast DMAs) ----
    # Encoder inputs are loaded first (inside the bc loop below) so the PE can
    # start transposing immediately; Wk is needed ~2us in, Wq/Wo much later.
    wq = wres.tile([P, ND * D], BF16, tag="wq")
    wo = wres.tile([P, ND * D], BF16, tag="wo")
    # Wk/Wv live in ao-tagged tiles (dead after the encoder phase)
    wk_lo = ao_p.tile([P, ND * SL], BF16, tag="ao", name="wk_lo")
    wk_hi = ao_p.tile([P, ND * SL], BF16, tag="ao", name="wk_hi")
    wv_lo = ao_p.tile([P, ND * SL], BF16, tag="ao", name="wv_lo")
    wv_hi = ao_p.tile([P, ND * SL], BF16, tag="ao", name="wv_hi")

    def wslice(lo, hi, i, c0, c1):
        t = lo if i < 4 else hi
        return t[:, D * (i % 4) + c0 : D * (i % 4) + c1]

    # ---- encoder phase: kT (scaled) + ksb + v0/v1 for all 8 bc ----
    kt, v0, v1, ksb = {}, {}, {}, {}
    encs = []
    for bc in range(BC):
        en0 = enc_p.tile([P, D], BF16, tag="en0", name=f"en0_{bc}", bufs=2)
        en1 = enc_p.tile([E1, D], BF16, tag="en1", name=f"en1_{bc}", bufs=2)
        nc.gpsimd.dma_start(out=en0, in_=d["enc"][bc, 0:E0, :])
        nc.gpsimd.dma_start(out=en1, in_=d["enc"][bc, E0:E, :])
        encs.append((en0, en1))
        if bc == 1:
            nc.gpsimd.dma_start(out=wk_lo, in_=d["Wk"].rearrange("(n p) d -> p n d", p=P)[:, 0:4, :])
            nc.gpsimd.dma_start(out=wk_hi, in_=d["Wk"].rearrange("(n p) d -> p n d", p=P)[:, 4:8, :])
            nc.gpsimd.dma_start(out=wv_lo, in_=d["Wv"].rearrange("(n p) d -> p n d", p=P)[:, 0:4, :])
            nc.gpsimd.dma_start(out=wv_hi, in_=d["Wv"].rearrange("(n p) d -> p n d", p=P)[:, 4:8, :])
    nc.gpsimd.dma_start(out=wq, in_=d["Wq"].rearrange("(n p) d -> p n d", p=P))
    nc.gpsimd.dma_start(out=wo, in_=d["Wo"].rearrange("(n p) d -> p n d", p=P))
    for bc in range(BC):
        en0, en1 = encs[bc]

        enct = []
        for i in range(ND):
            pst = psO.tile([P, 512], F32, tag="ps", name="pst").bitcast(BF16)[:, 0:E]
            sl = slice(128 * i, 128 * (i + 1))
            nc.tensor.transpose(pst[:, 0:E0], en0[:, sl], ident)
            nc.tensor.transpose(pst[:, E0:E], en1[:, sl], ident[0:E1, 0:E1])
            t = enct_p.tile([P, E], BF16, tag=f"e{i}", name=f"e{i}_{bc}")
            nc.scalar.activation(t, pst, COPY)
            enct.append(t)

        ktile = kv_p.tile([P, ND * E], BF16, tag=f"kt{bc}", name=f"kt{bc}")
        kt[bc] = ktile
        for j in range(ND):
            ps = psA.tile([P, 512], F32, tag="ps")
            for i in range(ND):
                nc.tensor.matmul(
                    ps[:, 0:E],
                    lhsT=wslice(wk_lo, wk_hi, i, 128 * j, 128 * (j + 1)),
                    rhs=enct[i],
                    start=(i == 0),
                    stop=(i == ND - 1),
                )
            ksl = ktile[:, E * j : E * (j + 1)]
            nc.scalar.activation(ksl, ps[:, 0:E], COPY, scale=SCALE)
            kb = kv_p.tile([P, 16], BF16, tag=f"ksb{bc}_{j}", name=f"ksb{bc}_{j}")
            nc.gpsimd.memset(kb, 0.0)
            # head 2j ksum -> col 15-2j (rows 0:64); head 2j+1 -> col 14-2j
            nc.vector.tensor_reduce(
                kb[0:64, 15 - 2 * j : 16 - 2 * j], ksl[0:64, :],
                axis=mybir.AxisListType.X, op=mybir.AluOpType.add,
            )
            nc.vector.tensor_reduce(
                kb[64:128, 14 - 2 * j : 15 - 2 * j], ksl[64:128, :],
                axis=mybir.AxisListType.X, op=mybir.AluOpType.add,
            )
            ksb[(bc, j)] = kb

        v0t = kv_p.tile([P, D], BF16, tag=f"v0_{bc}", name=f"v0_{bc}")
        v1t = kv_p.tile([P, D], BF16, tag=f"v1_{bc}", name=f"v1_{bc}")
        v0[bc], v1[bc] = v0t, v1t
        for half in range(2):
            cols = slice(512 * half, 512 * (half + 1))
            ps0 = psEAV.tile([P, 512], F32, tag="ps")
            ps1 = psEAV.tile([P, 512], F32, tag="ps")
            for i in range(ND):
                nc.tensor.matmul(
                    ps0, lhsT=enct[i][:, 0:E0], rhs=wslice(wv_lo, wv_hi, i, 512 * half, 512 * (half + 1)),
                    start=(i == 0), stop=(i == ND - 1),
                )
            for i in range(ND):
                nc.tensor.matmul(
                    ps1[0:E1, :], lhsT=enct[i][:, E0:E], rhs=wslice(wv_lo, wv_hi, i, 512 * half, 512 * (half + 1)),
                    start=(i == 0), stop=(i == ND - 1),
                )
            nc.scalar.activation(v0t[:, cols], ps0, COPY)
            # replicate v1 rows at partition bases 0 and 64
            nc.scalar.activation(v1t[0:E1, cols], ps1[0:E1, :], COPY)
            nc.vector.tensor_copy(v1t[64 : 64 + E1, cols], ps1[0:E1, :])

    # ---- main loop ----
    def emit_A(b, c):
        """x load + transpose + Q-projection for one component."""
        bc = c * B + b
        xin = xin_p.tile([P, 4 * D], BF16, tag="xin")
        nc.gpsimd.dma_start(out=xin, in_=d["x"][bc].rearrange("(k p) d -> p k d", p=P))
        xt = []
        for i in range(ND):
            pst = psO.tile([P, 512], F32, tag="ps", name="pst").bitcast(BF16)[:, 0:SL]
            for k in range(4):
                nc.tensor.transpose(
                    pst[:, 128 * k : 128 * (k + 1)],
                    xin[:, D * k + 128 * i : D * k + 128 * (i + 1)],
                    ident,
                )
            t = xt_p.tile([P, SL], BF16, tag=f"xt{i}", name=f"xt{i}_{bc}")
            nc.scalar.activation(t, pst, COPY)
            xt.append(t)
        qt = qt_p.tile([P, ND * SL], BF16, tag="qt")
        for j in range(ND):
            ps = psO.tile([P, 512], F32, tag="ps")
            for i in range(ND):
                nc.tensor.matmul(
                    ps,
                    lhsT=wq[:, D * i + 128 * j : D * i + 128 * (j + 1)],
                    rhs=xt[i],
                    start=(i == 0),
                    stop=(i == ND - 1),
                )
            nc.scalar.activation(qt[:, SL * j : SL * (j + 1)], ps, COPY)
        return qt

    def emit_B(b, c, qt, pl_ps, dn_ps, ao):
        """Scores + exp + AV + pooled-mm + den-mm for one component.

        Pooled/den stacks live at partition block 32c (rows 32c .. 32c+16);
        head h of the component sits at partition 32c + 15-h.  Block 96 needs
        explicit tile_position (the inferred path rejects base 96).
        """
        bc = c * B + b
        ktile = kt[bc]
        blk = 32 * c
        for j in range(ND):
            nc.tensor.matmul(
                pl_ps[blk : blk + 16, :],
                lhsT=ksb[(bc, j)],
                rhs=qt[:, SL * j : SL * (j + 1)],
                start=(j == 0),
                stop=(j == ND - 1),
                skip_group_check=True,
                tile_position=(0, blk),
            )
        for j in range(ND):  # head pairs (2j, 2j+1)
            psb = psEAV.tile([P, 512], F32, tag="ps", name="psb")
            was = []
            for hp in range(2):
                h = 2 * j + hp
                hr = 64 * hp
                qsl = qt[hr : hr + 64, SL * j : SL * (j + 1)]
                ps_a = psA.tile([P, 512], F32, tag="ps", name="ps_a")
                nc.tensor.matmul(
                    ps_a, lhsT=ktile[hr : hr + 64, E * j : E * j + E0], rhs=qsl,
                    start=True, stop=True,
                )
                nc.tensor.matmul(
                    psb[64 * hp : 64 * hp + E1, :],
                    lhsT=ktile[hr : hr + 64, E * j + E0 : E * j + E],
                    rhs=qsl,
                    start=True, stop=True, skip_group_check=True,
                )
                wa = wa_p.tile([P, SL], BF16, tag=f"wa{hp}", name=f"wa{hp}")
                nc.scalar.activation(wa, ps_a, EXP)
                was.append(wa)
            wb = wb_p.tile([P, SL], BF16, tag="wb", name="wb")
            nc.scalar.activation(wb[0 : 64 + E1, :], psb[0 : 64 + E1, :], EXP)
            ps_av = psEAV.tile([P, 512], F32, tag="ps", name="ps_av")
            for hp in range(2):
                h = 2 * j + hp
                hr = 64 * hp
                wa = was[hp]
                wbs = wb[64 * hp : 64 * hp + E1, :]
                vsl = slice(64 * h, 64 * (h + 1))
                nc.tensor.matmul(
                    ps_av[hr : hr + 64, :], lhsT=v0[bc][:, vsl], rhs=wa,
                    start=True, stop=False, skip_group_check=True,
                )
                nc.tensor.matmul(
                    ps_av[hr : hr + 64, :],
                    lhsT=v1[bc][64 * hp : 64 * hp + E1, vsl],
                    rhs=wbs,
                    start=False, stop=True, skip_group_check=True,
                )
                # denominator stack: den_h -> partition 32c + 15-h
                nc.tensor.matmul(
                    dn_ps[blk : blk + 16 - h, :],
                    lhsT=zden[:, h:16], rhs=wa,
                    start=(h == 0), stop=False,
                    skip_group_check=True,
                    tile_position=(0, blk),
                )
                nc.tensor.matmul(
                    dn_ps[blk : blk + 16 - h, :],
                    lhsT=zden[64 * hp : 64 * hp + E1, h:16],
                    rhs=wbs,
                    start=False, stop=(h == H - 1),
                    skip_group_check=True,
                    tile_position=(64 * hp, blk),
                )
            nc.vector.tensor_copy(ao[:, SL * j : SL * (j + 1)], ps_av)

    def emit_C(pl_ps, dn_ps):
        """coef[c] = softmax_c(exp(pooled/E)) / den, rows in 15-h order."""
        ep = [st_p.tile([16, SL], BF16, tag=f"ep{c}", name=f"ep{c}") for c in range(C)]
        rd = [st_p.tile([16, SL], BF16, tag=f"rd{c}", name=f"rd{c}") for c in range(C)]
        for c in range(C):
            nc.scalar.activation(ep[c], pl_ps[32 * c : 32 * c + 16, :], EXP, scale=1.0 / E)
            nc.vector.reciprocal(rd[c], dn_ps[32 * c : 32 * c + 16, :])
        sc = st_p.tile([16, SL], BF16, tag="sc", name="sc")
        nc.vector.tensor_add(sc, ep[0], ep[1])
        nc.vector.tensor_add(sc, sc, ep[2])
        nc.vector.tensor_add(sc, sc, ep[3])
        rs = st_p.tile([16, SL], BF16, tag="rs", name="rs")
        nc.vector.reciprocal(rs, sc)
        coefs = []
        for c in range(C):
            cf = st_p.tile([16, SL], BF16, tag=f"cf{c}", name=f"cf{c}")
            nc.vector.tensor_mul(rd[c], rd[c], rs)
            nc.vector.tensor_mul(cf, ep[c], rd[c])
            coefs.append(cf)
        return coefs

    def emit_D(aos, coefs):
        """ao *= broadcast(coef): PE selector matmul + DVE mul (in2 = PSUM)."""
        for c in range(C):
            for j in range(ND):
                cb = psO.tile([P, 512], F32, tag="ps", name="cb")
                nc.tensor.matmul(cb, lhsT=sel[j], rhs=coefs[c], start=True, stop=True)
                sl_ao = aos[c][:, SL * j : SL * (j + 1)]
                nc.vector.tensor_mul(sl_ao, sl_ao, cb)

    def emit_E(b, aos):
        """O-projection + bias + residual + store."""
        for c in range(C):
            bc = c * B + b
            for m in range(4):
                xr = xr_p.tile([P, D], F32, tag="xr")
                nc.sync.dma_start(out=xr, in_=d["x"][bc, 128 * m : 128 * (m + 1), :])
                oh = oh_p.tile([P, D], F32, tag="oh")
                pss = [psO.tile([P, 512], F32, tag="ps", name=f"pso{h}") for h in range(2)]
                for half in range(2):
                    nc.tensor.matmul(
                        pss[half], lhsT=ones1, rhs=bo_bf[:, 512 * half : 512 * (half + 1)],
                        start=True, stop=False, skip_group_check=True,
                    )
                for i in range(ND):
                    lhsT = aos[c][:, SL * i + 128 * m : SL * i + 128 * (m + 1)]
                    for half in range(2):
                        nc.tensor.matmul(
                            pss[half],
                            lhsT=lhsT,
                            rhs=wo[:, D * i + 512 * half : D * i + 512 * (half + 1)],
                            start=False,
                            stop=(i == ND - 1),
                            skip_group_check=True,
                        )
                for half in range(2):
                    cols = slice(512 * half, 512 * (half + 1))
                    nc.vector.tensor_add(oh[:, cols], pss[half], xr[:, cols])
                nc.sync.dma_start(out=d["out"][bc, 128 * m : 128 * (m + 1), :], in_=oh)

    pend = None
    for b in range(B):
        pl_ps = psPL.tile([P, 512], F32, tag="ps", name=f"pl{b}")
        dn_ps = psDN.tile([P, 512], F32, tag="ps", name=f"dn{b}")
        aos = {}
        for c in range(C):
            qt = emit_A(b, c)
            if c == 1 and pend is not None:
                emit_D(pend[1], pend[2])
                emit_E(pend[0], pend[1])
                pend = None
            aos[c] = ao_p.tile([P, ND * SL], BF16, tag="ao", name=f"ao{c}_{b}")
            emit_B(b, c, qt, pl_ps, dn_ps, aos[c])
        coefs = emit_C(pl_ps, dn_ps)
        pend = (b, aos, coefs)
    emit_D(pend[1], pend[2])
    emit_E(pend[0], pend[1])


def build_program(s_loc=S_LOC, n_cores=N_CORES):
    nc = bacc.Bacc(trn_type="TRN2", target_bir_lowering=False, debug=False, num_devices=n_cores)
    d = {
        "x": nc.dram_tensor("x", [BC, s_loc, D], F32, kind="ExternalInput").ap(),
        "enc": nc.dram_tensor("enc", [BC, E, D], F32, kind="ExternalInput").ap(),
        "Wq": nc.dram_tensor("Wq", [D, D], F32, kind="ExternalInput").ap(),
        "Wk": nc.dram_tensor("Wk", [D, D], F32, kind="ExternalInput").ap(),
        "Wv": nc.dram_tensor("Wv", [D, D], F32, kind="ExternalInput").ap(),
        "Wo": nc.dram_tensor("Wo", [D, D], F32, kind="ExternalInput").ap(),
        "bo": nc.dram_tensor("bo", [1, D], F32, kind="ExternalInput").ap(),
        "out": nc.dram_tensor("out", [BC, s_loc, D], F32, kind="ExternalOutput").ap(),
    }
    with TileContext(nc, trace_sim=False) as tc, ExitStack() as ctx:
        build_body(ctx, tc, d, s_loc)
    nc.compile()
    return nc


def make_in_maps(hidden_states, encoder_hidden_states, Wq, Wk, Wv, Wo, bo, s_loc=S_LOC, n_cores=N_CORES):
    common = {
        "enc": np.ascontiguousarray(encoder_hidden_states, dtype=np.float32),
        "Wq": np.ascontiguousarray(Wq, dtype=np.float32),
        "Wk": np.ascontiguousarray(Wk, dtype=np.float32),
        "Wv": np.ascontiguousarray(Wv, dtype=np.float32),
        "Wo": np.ascontiguousarray(Wo, dtype=np.float32),
        "bo": np.ascontiguousarray(bo, dtype=np.float32).reshape(1, D),
    }
    return [
        {"x": np.ascontiguousarray(hidden_states[:, i * s_loc : (i + 1) * s_loc, :], dtype=np.float32), **common}
        for i in range(n_cores)
    ]


_NC = None


def kernel(hidden_states, encoder_hidden_states, Wq, Wk, Wv, Wo, bo):
    global _NC
    if _NC is None:
        _NC = build_program()
    in_maps = make_in_maps(hidden_states, encoder_hidden_states, Wq, Wk, Wv, Wo, bo)
    res = run_bass_kernel_spmd(_NC, in_maps, list(range(N_CORES))).results
    out = np.concatenate([res[i]["out"] for i in range(N_CORES)], axis=1)
    return np.ascontiguousarray(out, dtype=np.float32)


if __name__ == "__main__":
    build_program()
    print("compile OK")
